# revision 11
# baseline (speedup 1.0000x reference)
"""Trainium2 Bass kernel for nn_BotRGCN2 (2-layer RGCN over 100k nodes / 600k edges).

Strategy (8 NeuronCores, SPMD):
  - Shard nodes across cores (12500/core, padded to 12544 = 98 windows of 128).
  - Feature-major (transposed) activations on-chip; node-major gather tables in DRAM.
  - Per RGCN layer: each core computes xw_r = x @ W_r for its own nodes
    (node-major), AllGather -> full 200704-row table in DRAM; then for each
    owned 128-node window, dma_gather the per-edge source rows (transform-first
    messages) and scatter-add them via one-hot matmuls on the PE
    (lhsT = gathered rows [128 edges x 128 feat], rhs = one-hot
    [128 edges x 128 window-slots] built by a single fused DVE tensor_scalar:
    (iota == dst_local) * (1/cnt)).  Mean-per-relation is folded into the
    per-edge weight; the root term is one more accumulating matmul per window.
  - Edges are preprocessed on the host: partitioned by dst owner, grouped by
    (window, src-owner-block) so every dma_gather instruction indexes a single
    <=25088-row table block (int16 index limit), padded to 128-edge chunks with
    weight-0 entries, with a chunk structure shared by all 8 cores so a single
    SPMD program works.
"""

import sys
from contextlib import ExitStack

import numpy as np

sys.path.insert(0, "/opt/trn_rl_repo")

import ml_dtypes  # noqa: E402
import concourse.bass as bass  # noqa: E402
import concourse.bacc as bacc  # noqa: E402
import concourse.mybir as mybir  # noqa: E402
import concourse.tile as tile  # noqa: E402
from concourse import library_config  # noqa: E402
from concourse.bass_utils import run_bass_kernel_spmd  # noqa: E402

C = 8           # cores
WIN = 128       # dst nodes per PSUM window
R = 2           # relations

# tunables
SG_WINDOWS = 8  # windows per gather supergroup
NIDX_CHUNKS_MAX = 8  # max 128-idx chunks per dma_gather (>1024 idxs hangs SDMA)
SINGLE_PACKET = True
G_BUFS = 0      # 0 = auto (max gather insts per supergroup + headroom)
S_BUFS = 6
PS_BUFS = 4
USE_BF16 = False
TRACE = False
TMPDIR = None

F32 = mybir.dt.float32
LAST_RESULTS = None  # BassKernelResults of the most recent run (for profiling)


def _dt():
    return mybir.dt.bfloat16 if USE_BF16 else mybir.dt.float32


def _np_dt():
    return ml_dtypes.bfloat16 if USE_BF16 else np.float32


def _col_tiles(total, width):
    out = []
    c = 0
    while c < total:
        out.append((c, min(width, total - c)))
        c += width
    return out


# ----------------------------------------------------------------------------
# host-side edge preprocessing
# ----------------------------------------------------------------------------

def _edge_meta(src, dst, et, N, NPC, NP2):
    """Build the SPMD-uniform chunk structure + per-core streams."""
    E = src.shape[0]
    NW = NP2 // WIN
    seg = dst * R + et
    cnt = np.bincount(seg, minlength=N * R).astype(np.float64)
    w = (1.0 / np.maximum(cnt, 1.0))[seg]

    core = dst // NPC
    nl = dst % NPC
    vwin = nl // WIN
    dloc = nl % WIN
    blk = src // NPC                       # table block == src owner core
    tloc = et * NP2 + (src % NPC)          # row within block (< 2*NP2 <= 32767)
    assert 2 * NP2 <= 32768

    counts = np.zeros((C, NW, C), np.int64)
    np.add.at(counts, (core, vwin, blk), 1)
    K = -(-counts.max(axis=0) // WIN)      # [NW, C] chunks per (win, blk), shared

    # compute-order chunk bases (v-major, then b, then k)
    co_base = np.zeros((NW, C), np.int64)
    cc = 0
    for v in range(NW):
        for b in range(C):
            co_base[v, b] = cc
            cc += K[v, b]
    TC = int(cc)

    # gather-order (supergroup, block, window, k) + gather instruction list
    go_base = np.zeros((NW, C), np.int64)
    gather_insts = []  # (blk, start_chunk, n_chunks, sg_start)
    gc = 0
    for s0 in range(0, NW, SG_WINDOWS):
        vs = range(s0, min(s0 + SG_WINDOWS, NW))
        for b in range(C):
            nch = int(sum(int(K[v, b]) for v in vs))
            if nch == 0:
                continue
            off = 0
            while off < nch:
                n = min(NIDX_CHUNKS_MAX, nch - off)
                gather_insts.append((b, gc + off, n, s0))
                off += n
            for v in vs:
                go_base[v, b] = gc
                gc += K[v, b]
    assert gc == TC

    # per-core streams
    order = np.argsort((core * NW + vwin) * C + blk, kind="stable")
    gid = ((core * NW + vwin) * C + blk)[order]
    starts = np.concatenate([[0], np.cumsum(np.bincount(gid, minlength=C * NW * C))])
    rank = np.arange(E) - starts[gid]

    ce = core[order]
    v_ = vwin[order]
    b_ = blk[order]
    k_ = rank // WIN
    lane = rank % WIN

    idxg = np.zeros((C, TC * WIN), np.int16)
    md = np.zeros((C, TC * WIN), np.float32)
    mwv = np.zeros((C, TC * WIN), np.float32)
    cpos = (co_base[v_, b_] + k_) * WIN + lane
    gpos = (go_base[v_, b_] + k_) * WIN + lane
    idxg[ce, gpos] = tloc[order].astype(np.int16)
    md[ce, cpos] = dloc[order]
    mwv[ce, cpos] = w[order]

    # wrap indices per gather instruction: idx i -> [i%16, off + i//16]
    TIDX = TC * WIN
    idxw = np.zeros((C, 128, TIDX // 16), np.int16)
    for (b, gc0, nch, s0) in gather_insts:
        n = nch * WIN
        segm = idxg[:, gc0 * WIN: gc0 * WIN + n].reshape(C, n // 16, 16)
        idxw[:, :16, gc0 * 8: gc0 * 8 + n // 16] = segm.transpose(0, 2, 1)
    idxw[:, 16:, :] = np.tile(idxw[:, :16, :], (1, 7, 1))

    md = md.reshape(C, TC, WIN).transpose(0, 2, 1)   # [C, 128, TC]
    mwv = mwv.reshape(C, TC, WIN).transpose(0, 2, 1)

    return dict(K=K, co_base=co_base, go_base=go_base, gather_insts=gather_insts,
                TC=TC, TIDX=TIDX, NW=NW, idxw=idxw, md=md, mw=mwv)


# ----------------------------------------------------------------------------
# device program
# ----------------------------------------------------------------------------

def _build_program(shapes, meta):
    DT = _dt()
    N, TW, D, OUT, NPC, NP2 = (shapes[k] for k in
                               ("N", "TW", "D", "OUT", "NPC", "NP2"))
    KT = TW // 128
    NW = meta["NW"]
    TC, TIDX = meta["TC"], meta["TIDX"]
    K, co_base, go_base = meta["K"], meta["co_base"], meta["go_base"]
    gather_insts = meta["gather_insts"]
    BR = R * NP2
    AF = mybir.ActivationFunctionType
    ALU = mybir.AluOpType

    nc = bacc.Bacc("TRN2", target_bir_lowering=False)

    twT = nc.dram_tensor("twT", [KT, 128, NP2], DT, kind="ExternalInput")
    idx16 = nc.dram_tensor("idx16", [128, TIDX // 16], mybir.dt.int16,
                           kind="ExternalInput")
    mdst = nc.dram_tensor("mdst", [128, TC], DT, kind="ExternalInput")
    mw = nc.dram_tensor("mw", [128, TC], DT, kind="ExternalInput")
    wt = nc.dram_tensor("wt", [128, KT, 128], DT, kind="ExternalInput")
    bt = nc.dram_tensor("bt", [128, 1], F32, kind="ExternalInput")
    win = nc.dram_tensor("win", [128, 128], DT, kind="ExternalInput")
    bin_ = nc.dram_tensor("bin", [128, 1], F32, kind="ExternalInput")
    wr = nc.dram_tensor("wr", [128, R * 128], DT, kind="ExternalInput")
    root = nc.dram_tensor("root", [128, 128], DT, kind="ExternalInput")
    brg = nc.dram_tensor("brg", [128, 1], F32, kind="ExternalInput")
    w1 = nc.dram_tensor("w1", [128, 128], DT, kind="ExternalInput")
    b1 = nc.dram_tensor("b1", [128, 1], F32, kind="ExternalInput")
    w2 = nc.dram_tensor("w2", [128, OUT], DT, kind="ExternalInput")
    b2 = nc.dram_tensor("b2", [OUT, 1], F32, kind="ExternalInput")
    iota = nc.dram_tensor("iota", [128, WIN], DT, kind="ExternalInput")
    outT = nc.dram_tensor("outT", [OUT, NP2], F32, kind="ExternalOutput")

    with tile.TileContext(nc) as tc:
        nc.gpsimd.load_library(library_config.mlp)
        with ExitStack() as stack:
            cpool = stack.enter_context(tc.tile_pool(name="const", bufs=1))
            dpool = stack.enter_context(
                tc.tile_pool(name="dram", bufs=1, space="DRAM"))
            persist = stack.enter_context(tc.tile_pool(name="persist", bufs=1))

            def cload(dram_t, shape, dtype):
                t = cpool.tile(shape, dtype, name=f"c_{dram_t.name}")
                nc.sync.dma_start(t[:], dram_t[:])
                return t

            wt_s = cload(wt, [128, KT, 128], DT)
            bt_s = cload(bt, [128, 1], F32)
            win_s = cload(win, [128, 128], DT)
            bin_s = cload(bin_, [128, 1], F32)
            wr_s = cload(wr, [128, R * 128], DT)
            root_s = cload(root, [128, 128], DT)
            brg_s = cload(brg, [128, 1], F32)
            w1_s = cload(w1, [128, 128], DT)
            b1_s = cload(b1, [128, 1], F32)
            w2_s = cload(w2, [128, OUT], DT)
            b2_s = cload(b2, [OUT, 1], F32)
            iota_s = cload(iota, [128, WIN], DT)
            idx_s = cload(idx16, [128, TIDX // 16], mybir.dt.int16)
            mdst_s = cload(mdst, [128, TC], DT)
            mw_s = cload(mw, [128, TC], DT)

            tables = [dpool.tile([C * BR, 128], DT, addr_space="Shared",
                                 name=f"table{i}") for i in range(2)]
            agin = dpool.tile([R, NP2, 128], DT, name="agin")

            xT = persist.tile([128, NP2], DT, name="xT")

            # ---------------- stage 1: x = lrelu(lrelu(tweet@Wt+bt)@Win+bin)
            with tc.tile_pool(name="s1", bufs=3) as s1p, \
                 tc.tile_pool(name="ps1", bufs=2, space="PSUM") as ps1:
                for (c0, fw) in _col_tiles(NP2, 512):
                    twt = s1p.tile([128, KT, fw], DT, tag="twt", name="twt")
                    nc.sync.dma_start(
                        twt[:], twT[:, :, c0:c0 + fw].rearrange("k p f -> p k f"))
                    ps_t = ps1.tile([128, fw], F32, tag="pst", name="ps_t")
                    for k in range(KT):
                        nc.tensor.matmul(ps_t[:], wt_s[:, k, :], twt[:, k, :],
                                         start=(k == 0), stop=(k == KT - 1))
                    tt = s1p.tile([128, fw], DT, tag="tt", name="tt")
                    nc.scalar.activation(tt[:], ps_t[:], AF.Lrelu,
                                         bias=bt_s[:], alpha=0.01)
                    ps_x = ps1.tile([128, fw], F32, tag="psx", name="ps_x")
                    nc.tensor.matmul(ps_x[:], win_s[:], tt[:],
                                     start=True, stop=True)
                    nc.scalar.activation(xT[:, c0:c0 + fw], ps_x[:], AF.Lrelu,
                                         bias=bin_s[:], alpha=0.01)

            # ---------------- 2 RGCN layers
            for layer in range(2):
                table = tables[layer]
                # phase A: local xw table shard + AllGather
                with tc.tile_pool(name=f"pa{layer}", bufs=3) as pap, \
                     tc.tile_pool(name=f"psa{layer}", bufs=2,
                                  space="PSUM") as psa:
                    for nt in range(NW):
                        psA = psa.tile([128, R * 128], F32, tag="psA",
                                       name="psA")
                        nc.tensor.matmul(psA[:], xT[:, nt * 128:(nt + 1) * 128],
                                         wr_s[:], start=True, stop=True)
                        ob = pap.tile([128, R, 128], DT, tag="ob", name="ob")
                        nc.scalar.activation(
                            ob[:].rearrange("p e f -> p (e f)"), psA[:],
                            AF.Copy)
                        nc.sync.dma_start(
                            agin[:, nt * 128:(nt + 1) * 128, :]
                            .rearrange("e n f -> n e f"), ob[:])
                    nc.gpsimd.collective_compute(
                        "AllGather", mybir.AluOpType.bypass,
                        replica_groups=[list(range(C))],
                        ins=[agin[:].rearrange("e n f -> (e n) f")],
                        outs=[table[:]])

                # phase B: gather + one-hot scatter matmuls per window
                per_sg = {}
                for (b, gc0, nch, s0) in gather_insts:
                    per_sg[s0] = per_sg.get(s0, 0) + 1
                g_bufs = G_BUFS or (max(per_sg.values()) + 4)
                with tc.tile_pool(name=f"g{layer}", bufs=g_bufs) as gp, \
                     tc.tile_pool(name=f"s{layer}", bufs=S_BUFS) as sp, \
                     tc.tile_pool(name=f"pb{layer}", bufs=PS_BUFS,
                                  space="PSUM") as pb:
                    by_sg = {}
                    for (b, gc0, nch, s0) in gather_insts:
                        by_sg.setdefault(s0, []).append((b, gc0, nch))
                    for s0 in range(0, NW, SG_WINDOWS):
                        vs = range(s0, min(s0 + SG_WINDOWS, NW))
                        gts = {}
                        for (b, gc0, nch) in by_sg.get(s0, []):
                            gt = gp.tile([128, nch, 128], DT, tag="g",
                                         name="gt")
                            nc.gpsimd.dma_gather(
                                gt[:], table[b * BR:(b + 1) * BR, :],
                                idx_s[:, gc0 * 8: (gc0 + nch) * 8],
                                nch * WIN, nch * WIN, 128,
                                single_packet=SINGLE_PACKET)
                            gts.setdefault(b, []).append((gt, gc0, nch))
                        for v in vs:
                            ps = pb.tile([128, WIN], F32, tag="psb", name="psb")
                            i = 0
                            for b in range(C):
                                for k in range(int(K[v, b])):
                                    ccx = int(co_base[v, b]) + k
                                    st = sp.tile([128, WIN], DT, tag="s",
                                                 name="st")
                                    nc.vector.tensor_scalar(
                                        st[:], iota_s[:],
                                        mdst_s[:, ccx:ccx + 1],
                                        mw_s[:, ccx:ccx + 1],
                                        op0=ALU.is_equal, op1=ALU.mult)
                                    cg = int(go_base[v, b]) + k
                                    gt = None
                                    for (g_t, g_0, g_n) in gts[b]:
                                        if g_0 <= cg < g_0 + g_n:
                                            gt, j = g_t, cg - g_0
                                            break
                                    nc.tensor.matmul(ps[:], gt[:, j, :], st[:],
                                                     start=(i == 0), stop=False)
                                    i += 1
                            nc.tensor.matmul(ps[:], root_s[:],
                                             xT[:, v * 128:(v + 1) * 128],
                                             start=(i == 0), stop=True)
                            nc.vector.tensor_scalar(
                                xT[:, v * 128:(v + 1) * 128], ps[:],
                                brg_s[:], None, op0=ALU.add)

            # ---------------- head
            with tc.tile_pool(name="hd", bufs=3) as hp, \
                 tc.tile_pool(name="psh", bufs=2, space="PSUM") as psh, \
                 tc.tile_pool(name="outp", bufs=1) as outp:
                outT_s = outp.tile([OUT, NP2], F32, name="outT_s")
                for (c0, fw) in _col_tiles(NP2, 512):
                    ph = psh.tile([128, fw], F32, tag="ph", name="ph")
                    nc.tensor.matmul(ph[:], w1_s[:], xT[:, c0:c0 + fw],
                                     start=True, stop=True)
                    ht = hp.tile([128, fw], DT, tag="ht", name="ht")
                    nc.scalar.activation(ht[:], ph[:], AF.Lrelu,
                                         bias=b1_s[:], alpha=0.01)
                    po = psh.tile([OUT, fw], F32, tag="po", name="po")
                    nc.tensor.matmul(po[:], w2_s[:], ht[:],
                                     start=True, stop=True)
                    nc.vector.tensor_scalar(outT_s[:, c0:c0 + fw], po[:],
                                            b2_s[:], None, op0=ALU.add)
                nc.sync.dma_start(outT[:, :], outT_s[:])

    nc.compile()
    return nc


# ----------------------------------------------------------------------------
# entry point
# ----------------------------------------------------------------------------

def kernel(**inputs):
    global LAST_RESULTS
    tweet = np.asarray(inputs["tweet"], np.float32)
    ei = np.asarray(inputs["edge_index"]).astype(np.int64)
    et = np.asarray(inputs["edge_type"]).astype(np.int64)
    W_tweet = np.asarray(inputs["W_tweet"], np.float32)
    b_tweet = np.asarray(inputs["b_tweet"], np.float32)
    W_in = np.asarray(inputs["W_in"], np.float32)
    b_in = np.asarray(inputs["b_in"], np.float32)
    rgcn_weight = np.asarray(inputs["rgcn_weight"], np.float32)
    rgcn_root = np.asarray(inputs["rgcn_root"], np.float32)
    rgcn_bias = np.asarray(inputs["rgcn_bias"], np.float32)
    W_out1 = np.asarray(inputs["W_out1"], np.float32)
    b_out1 = np.asarray(inputs["b_out1"], np.float32)
    W_out2 = np.asarray(inputs["W_out2"], np.float32)
    b_out2 = np.asarray(inputs["b_out2"], np.float32)

    N, TW = tweet.shape
    D = W_in.shape[0]
    OUT = W_out2.shape[1]
    assert N % C == 0 and TW % 128 == 0 and D == 128
    NPC = N // C
    NP2 = -(-NPC // WIN) * WIN
    src, dst = ei[0], ei[1]

    meta = _edge_meta(src, dst, et, N, NPC, NP2)
    shapes = dict(N=N, TW=TW, D=D, OUT=OUT, NPC=NPC, NP2=NP2)
    npdt = _np_dt()
    KT = TW // 128

    nc = _build_program(shapes, meta)

    # shared (replicated) weight tensors
    shared = {
        "wt": np.ascontiguousarray(
            W_tweet.reshape(KT, 128, 128).transpose(1, 0, 2)).astype(npdt),
        "bt": b_tweet.reshape(128, 1),
        "win": W_in.astype(npdt),
        "bin": b_in.reshape(128, 1),
        "wr": np.ascontiguousarray(
            rgcn_weight.transpose(1, 0, 2).reshape(128, R * 128)).astype(npdt),
        "root": rgcn_root.astype(npdt),
        "brg": rgcn_bias.reshape(128, 1),
        "w1": W_out1.astype(npdt),
        "b1": b_out1.reshape(128, 1),
        "w2": W_out2.astype(npdt),
        "b2": b_out2.reshape(OUT, 1),
        "iota": np.tile(np.arange(WIN, dtype=np.float32),
                        (128, 1)).astype(npdt),
    }

    in_maps = []
    for c in range(C):
        tw_c = np.zeros((KT, 128, NP2), npdt)
        tw_c[:, :, :NPC] = (tweet[c * NPC:(c + 1) * NPC].T
                            .reshape(KT, 128, NPC).astype(npdt))
        m = dict(shared)
        m["twT"] = tw_c
        m["idx16"] = meta["idxw"][c]
        m["mdst"] = meta["md"][c].astype(npdt)
        m["mw"] = meta["mw"][c].astype(npdt)
        in_maps.append(m)

    res = run_bass_kernel_spmd(nc, in_maps, core_ids=list(range(C)),
                               trace=TRACE, tmpdir=TMPDIR)
    LAST_RESULTS = res

    out = np.zeros((N, OUT), np.float32)
    for c in range(C):
        out[c * NPC:(c + 1) * NPC] = res.results[c]["outT"][:, :NPC].T
    return out


# revision 15
# speedup vs baseline: 1.1873x; 1.1873x over previous
"""Trainium2 Bass kernel for nn_BotRGCN2 (2-layer RGCN over 100k nodes / 600k edges).

Strategy (8 NeuronCores, SPMD):
  - Shard nodes across cores (12500/core, padded to 12544 = 98 windows of 128).
  - Feature-major (transposed) activations on-chip; node-major gather tables in DRAM.
  - Per RGCN layer: each core computes xw_r = x @ W_r for its own nodes
    (node-major), AllGather -> full 200704-row table in DRAM; then for each
    owned 128-node window, dma_gather the per-edge source rows (transform-first
    messages) and scatter-add them via one-hot matmuls on the PE
    (lhsT = gathered rows [128 edges x 128 feat], rhs = one-hot
    [128 edges x 128 window-slots] built by a single fused DVE tensor_scalar:
    (iota == dst_local) * (1/cnt)).  Mean-per-relation is folded into the
    per-edge weight; the root term is one more accumulating matmul per window.
  - Edges are preprocessed on the host: partitioned by dst owner, grouped by
    (window, src-owner-block) so every dma_gather instruction indexes a single
    <=25088-row table block (int16 index limit), padded to 128-edge chunks with
    weight-0 entries, with a chunk structure shared by all 8 cores so a single
    SPMD program works.
"""

import sys
from contextlib import ExitStack

import numpy as np

sys.path.insert(0, "/opt/trn_rl_repo")

import ml_dtypes  # noqa: E402
import concourse.bass as bass  # noqa: E402
import concourse.bacc as bacc  # noqa: E402
import concourse.mybir as mybir  # noqa: E402
import concourse.tile as tile  # noqa: E402
from concourse import library_config  # noqa: E402
from concourse.bass_utils import run_bass_kernel_spmd  # noqa: E402

C = 8           # cores
WIN = 128       # dst nodes per PSUM window
R = 2           # relations

# tunables
SG_WINDOWS = 12  # windows per gather supergroup
NIDX_CHUNKS_MAX = 8  # max 128-idx chunks per dma_gather (>1024 idxs hangs SDMA)
SINGLE_PACKET = True
G_BUFS = 0      # 0 = auto (max gather insts per supergroup + headroom)
S_BUFS = 6
PS_BUFS = 4
USE_BF16 = False
TRACE = False
TMPDIR = None

F32 = mybir.dt.float32
LAST_RESULTS = None  # BassKernelResults of the most recent run (for profiling)


def _dt():
    return mybir.dt.bfloat16 if USE_BF16 else mybir.dt.float32


def _np_dt():
    return ml_dtypes.bfloat16 if USE_BF16 else np.float32


def _col_tiles(total, width):
    out = []
    c = 0
    while c < total:
        out.append((c, min(width, total - c)))
        c += width
    return out


# ----------------------------------------------------------------------------
# host-side edge preprocessing
# ----------------------------------------------------------------------------

def _edge_meta(src, dst, et, N, NPC, NP2):
    """Build the SPMD-uniform chunk structure + per-core streams."""
    E = src.shape[0]
    NW = NP2 // WIN
    seg = dst * R + et
    cnt = np.bincount(seg, minlength=N * R).astype(np.float64)
    w = (1.0 / np.maximum(cnt, 1.0))[seg]

    core = dst // NPC
    nl = dst % NPC
    vwin = nl // WIN
    dloc = nl % WIN
    blk = src // NPC                       # table block == src owner core
    tloc = et * NP2 + (src % NPC)          # row within block (< 2*NP2 <= 32767)
    assert 2 * NP2 <= 32768

    counts = np.zeros((C, NW, C), np.int64)
    np.add.at(counts, (core, vwin, blk), 1)
    K = -(-counts.max(axis=0) // WIN)      # [NW, C] chunks per (win, blk), shared

    # compute-order chunk bases (v-major, then b, then k)
    co_base = np.zeros((NW, C), np.int64)
    cc = 0
    for v in range(NW):
        for b in range(C):
            co_base[v, b] = cc
            cc += K[v, b]
    TC = int(cc)

    # gather-order (supergroup, block, window, k) + gather instruction list
    go_base = np.zeros((NW, C), np.int64)
    gather_insts = []  # (blk, start_chunk, n_chunks, sg_start)
    gc = 0
    for s0 in range(0, NW, SG_WINDOWS):
        vs = range(s0, min(s0 + SG_WINDOWS, NW))
        for b in range(C):
            nch = int(sum(int(K[v, b]) for v in vs))
            if nch == 0:
                continue
            off = 0
            while off < nch:
                n = min(NIDX_CHUNKS_MAX, nch - off)
                gather_insts.append((b, gc + off, n, s0))
                off += n
            for v in vs:
                go_base[v, b] = gc
                gc += K[v, b]
    assert gc == TC

    # per-core streams
    order = np.argsort((core * NW + vwin) * C + blk, kind="stable")
    gid = ((core * NW + vwin) * C + blk)[order]
    starts = np.concatenate([[0], np.cumsum(np.bincount(gid, minlength=C * NW * C))])
    rank = np.arange(E) - starts[gid]

    ce = core[order]
    v_ = vwin[order]
    b_ = blk[order]
    k_ = rank // WIN
    lane = rank % WIN

    idxg = np.zeros((C, TC * WIN), np.int16)
    md = np.zeros((C, TC * WIN), np.float32)
    mwv = np.zeros((C, TC * WIN), np.float32)
    cpos = (co_base[v_, b_] + k_) * WIN + lane
    gpos = (go_base[v_, b_] + k_) * WIN + lane
    idxg[ce, gpos] = tloc[order].astype(np.int16)
    md[ce, cpos] = dloc[order]
    mwv[ce, cpos] = w[order]

    # wrap indices per gather instruction: idx i -> [i%16, off + i//16]
    TIDX = TC * WIN
    idxw = np.zeros((C, 128, TIDX // 16), np.int16)
    for (b, gc0, nch, s0) in gather_insts:
        n = nch * WIN
        segm = idxg[:, gc0 * WIN: gc0 * WIN + n].reshape(C, n // 16, 16)
        idxw[:, :16, gc0 * 8: gc0 * 8 + n // 16] = segm.transpose(0, 2, 1)
    idxw[:, 16:, :] = np.tile(idxw[:, :16, :], (1, 7, 1))

    md = md.reshape(C, TC, WIN).transpose(0, 2, 1)   # [C, 128, TC]
    mwv = mwv.reshape(C, TC, WIN).transpose(0, 2, 1)

    return dict(K=K, co_base=co_base, go_base=go_base, gather_insts=gather_insts,
                TC=TC, TIDX=TIDX, NW=NW, idxw=idxw, md=md, mw=mwv)


# ----------------------------------------------------------------------------
# device program
# ----------------------------------------------------------------------------

def _build_program(shapes, meta):
    DT = _dt()
    N, TW, D, OUT, NPC, NP2 = (shapes[k] for k in
                               ("N", "TW", "D", "OUT", "NPC", "NP2"))
    KT = TW // 128
    NW = meta["NW"]
    TC, TIDX = meta["TC"], meta["TIDX"]
    K, co_base, go_base = meta["K"], meta["co_base"], meta["go_base"]
    gather_insts = meta["gather_insts"]
    BR = R * NP2
    AF = mybir.ActivationFunctionType
    ALU = mybir.AluOpType

    nc = bacc.Bacc("TRN2", target_bir_lowering=False)

    twT = nc.dram_tensor("twT", [KT, 128, NP2], DT, kind="ExternalInput")
    idx16 = nc.dram_tensor("idx16", [128, TIDX // 16], mybir.dt.int16,
                           kind="ExternalInput")
    mdst = nc.dram_tensor("mdst", [128, TC], F32, kind="ExternalInput")
    mw = nc.dram_tensor("mw", [128, TC], F32, kind="ExternalInput")
    wt = nc.dram_tensor("wt", [128, KT, 128], DT, kind="ExternalInput")
    bt = nc.dram_tensor("bt", [128, 1], F32, kind="ExternalInput")
    win = nc.dram_tensor("win", [128, 128], DT, kind="ExternalInput")
    bin_ = nc.dram_tensor("bin", [128, 1], F32, kind="ExternalInput")
    wr = nc.dram_tensor("wr", [128, R * 128], DT, kind="ExternalInput")
    root = nc.dram_tensor("root", [128, 128], DT, kind="ExternalInput")
    brg = nc.dram_tensor("brg", [128, 1], F32, kind="ExternalInput")
    w1 = nc.dram_tensor("w1", [128, 128], DT, kind="ExternalInput")
    b1 = nc.dram_tensor("b1", [128, 1], F32, kind="ExternalInput")
    w2 = nc.dram_tensor("w2", [128, OUT], DT, kind="ExternalInput")
    b2 = nc.dram_tensor("b2", [OUT, 1], F32, kind="ExternalInput")
    iota = nc.dram_tensor("iota", [128, WIN], DT, kind="ExternalInput")
    outT = nc.dram_tensor("outT", [OUT, NP2], F32, kind="ExternalOutput")

    with tile.TileContext(nc) as tc:
        nc.gpsimd.load_library(library_config.mlp)
        with ExitStack() as stack:
            cpool = stack.enter_context(tc.tile_pool(name="const", bufs=1))
            dpool = stack.enter_context(
                tc.tile_pool(name="dram", bufs=1, space="DRAM"))
            persist = stack.enter_context(tc.tile_pool(name="persist", bufs=1))

            def cload(dram_t, shape, dtype):
                t = cpool.tile(shape, dtype, name=f"c_{dram_t.name}")
                nc.sync.dma_start(t[:], dram_t[:])
                return t

            wt_s = cload(wt, [128, KT, 128], DT)
            bt_s = cload(bt, [128, 1], F32)
            win_s = cload(win, [128, 128], DT)
            bin_s = cload(bin_, [128, 1], F32)
            wr_s = cload(wr, [128, R * 128], DT)
            root_s = cload(root, [128, 128], DT)
            brg_s = cload(brg, [128, 1], F32)
            w1_s = cload(w1, [128, 128], DT)
            b1_s = cload(b1, [128, 1], F32)
            w2_s = cload(w2, [128, OUT], DT)
            b2_s = cload(b2, [OUT, 1], F32)
            iota_s = cload(iota, [128, WIN], DT)
            idx_s = cload(idx16, [128, TIDX // 16], mybir.dt.int16)
            mdst_s = cload(mdst, [128, TC], F32)
            mw_s = cload(mw, [128, TC], F32)

            tables = [dpool.tile([C * BR, 128], DT, addr_space="Shared",
                                 name=f"table{i}") for i in range(2)]
            agin = dpool.tile([R, NP2, 128], DT, name="agin")

            xT = persist.tile([128, NP2], DT, name="xT")

            # ---------------- stage 1: x = lrelu(lrelu(tweet@Wt+bt)@Win+bin)
            with tc.tile_pool(name="s1", bufs=3) as s1p, \
                 tc.tile_pool(name="ps1", bufs=2, space="PSUM") as ps1:
                for (c0, fw) in _col_tiles(NP2, 512):
                    twt = s1p.tile([128, KT, fw], DT, tag="twt", name="twt")
                    nc.sync.dma_start(
                        twt[:], twT[:, :, c0:c0 + fw].rearrange("k p f -> p k f"))
                    ps_t = ps1.tile([128, fw], F32, tag="pst", name="ps_t")
                    for k in range(KT):
                        nc.tensor.matmul(ps_t[:], wt_s[:, k, :], twt[:, k, :],
                                         start=(k == 0), stop=(k == KT - 1))
                    tt = s1p.tile([128, fw], DT, tag="tt", name="tt")
                    nc.scalar.activation(tt[:], ps_t[:], AF.Lrelu,
                                         bias=bt_s[:], alpha=0.01)
                    ps_x = ps1.tile([128, fw], F32, tag="psx", name="ps_x")
                    nc.tensor.matmul(ps_x[:], win_s[:], tt[:],
                                     start=True, stop=True)
                    nc.scalar.activation(xT[:, c0:c0 + fw], ps_x[:], AF.Lrelu,
                                         bias=bin_s[:], alpha=0.01)

            # ---------------- 2 RGCN layers
            for layer in range(2):
                table = tables[layer]
                # phase A: local xw table shard + AllGather
                with tc.tile_pool(name=f"pa{layer}", bufs=3) as pap, \
                     tc.tile_pool(name=f"psa{layer}", bufs=2,
                                  space="PSUM") as psa:
                    for nt in range(NW):
                        psA = psa.tile([128, R * 128], F32, tag="psA",
                                       name="psA")
                        nc.tensor.matmul(psA[:], xT[:, nt * 128:(nt + 1) * 128],
                                         wr_s[:], start=True, stop=True)
                        ob = pap.tile([128, R, 128], DT, tag="ob", name="ob")
                        nc.scalar.activation(
                            ob[:].rearrange("p e f -> p (e f)"), psA[:],
                            AF.Copy)
                        nc.sync.dma_start(
                            agin[:, nt * 128:(nt + 1) * 128, :]
                            .rearrange("e n f -> n e f"), ob[:])
                    nc.gpsimd.collective_compute(
                        "AllGather", mybir.AluOpType.bypass,
                        replica_groups=[list(range(C))],
                        ins=[agin[:].rearrange("e n f -> (e n) f")],
                        outs=[table[:]])

                # phase B: gather + one-hot scatter matmuls per window
                per_sg = {}
                for (b, gc0, nch, s0) in gather_insts:
                    per_sg[s0] = per_sg.get(s0, 0) + 1
                g_bufs = G_BUFS or (max(per_sg.values()) + 4)
                with tc.tile_pool(name=f"g{layer}", bufs=g_bufs) as gp, \
                     tc.tile_pool(name=f"s{layer}", bufs=S_BUFS) as sp, \
                     tc.tile_pool(name=f"pb{layer}", bufs=PS_BUFS,
                                  space="PSUM") as pb:
                    by_sg = {}
                    for (b, gc0, nch, s0) in gather_insts:
                        by_sg.setdefault(s0, []).append((b, gc0, nch))
                    for s0 in range(0, NW, SG_WINDOWS):
                        vs = range(s0, min(s0 + SG_WINDOWS, NW))
                        gts = {}
                        for (b, gc0, nch) in by_sg.get(s0, []):
                            gt = gp.tile([128, nch, 128], DT, tag="g",
                                         name="gt")
                            nc.gpsimd.dma_gather(
                                gt[:], table[b * BR:(b + 1) * BR, :],
                                idx_s[:, gc0 * 8: (gc0 + nch) * 8],
                                nch * WIN, nch * WIN, 128,
                                single_packet=SINGLE_PACKET)
                            gts.setdefault(b, []).append((gt, gc0, nch))
                        for v in vs:
                            ps = pb.tile([128, WIN], F32, tag="psb", name="psb")
                            i = 0
                            for b in range(C):
                                for k in range(int(K[v, b])):
                                    ccx = int(co_base[v, b]) + k
                                    st = sp.tile([128, WIN], DT, tag="s",
                                                 name="st")
                                    nc.vector.tensor_scalar(
                                        st[:], iota_s[:],
                                        mdst_s[:, ccx:ccx + 1],
                                        mw_s[:, ccx:ccx + 1],
                                        op0=ALU.is_equal, op1=ALU.mult)
                                    cg = int(go_base[v, b]) + k
                                    gt = None
                                    for (g_t, g_0, g_n) in gts[b]:
                                        if g_0 <= cg < g_0 + g_n:
                                            gt, j = g_t, cg - g_0
                                            break
                                    nc.tensor.matmul(ps[:], gt[:, j, :], st[:],
                                                     start=(i == 0), stop=False)
                                    i += 1
                            nc.tensor.matmul(ps[:], root_s[:],
                                             xT[:, v * 128:(v + 1) * 128],
                                             start=(i == 0), stop=True)
                            nc.vector.tensor_scalar(
                                xT[:, v * 128:(v + 1) * 128], ps[:],
                                brg_s[:], None, op0=ALU.add)

            # ---------------- head
            with tc.tile_pool(name="hd", bufs=3) as hp, \
                 tc.tile_pool(name="psh", bufs=2, space="PSUM") as psh, \
                 tc.tile_pool(name="outp", bufs=1) as outp:
                outT_s = outp.tile([OUT, NP2], F32, name="outT_s")
                for (c0, fw) in _col_tiles(NP2, 512):
                    ph = psh.tile([128, fw], F32, tag="ph", name="ph")
                    nc.tensor.matmul(ph[:], w1_s[:], xT[:, c0:c0 + fw],
                                     start=True, stop=True)
                    ht = hp.tile([128, fw], DT, tag="ht", name="ht")
                    nc.scalar.activation(ht[:], ph[:], AF.Lrelu,
                                         bias=b1_s[:], alpha=0.01)
                    po = psh.tile([OUT, fw], F32, tag="po", name="po")
                    nc.tensor.matmul(po[:], w2_s[:], ht[:],
                                     start=True, stop=True)
                    nc.vector.tensor_scalar(outT_s[:, c0:c0 + fw], po[:],
                                            b2_s[:], None, op0=ALU.add)
                nc.sync.dma_start(outT[:, :], outT_s[:])

    nc.compile()
    return nc


# ----------------------------------------------------------------------------
# entry point
# ----------------------------------------------------------------------------

def kernel(**inputs):
    global LAST_RESULTS
    tweet = np.asarray(inputs["tweet"], np.float32)
    ei = np.asarray(inputs["edge_index"]).astype(np.int64)
    et = np.asarray(inputs["edge_type"]).astype(np.int64)
    W_tweet = np.asarray(inputs["W_tweet"], np.float32)
    b_tweet = np.asarray(inputs["b_tweet"], np.float32)
    W_in = np.asarray(inputs["W_in"], np.float32)
    b_in = np.asarray(inputs["b_in"], np.float32)
    rgcn_weight = np.asarray(inputs["rgcn_weight"], np.float32)
    rgcn_root = np.asarray(inputs["rgcn_root"], np.float32)
    rgcn_bias = np.asarray(inputs["rgcn_bias"], np.float32)
    W_out1 = np.asarray(inputs["W_out1"], np.float32)
    b_out1 = np.asarray(inputs["b_out1"], np.float32)
    W_out2 = np.asarray(inputs["W_out2"], np.float32)
    b_out2 = np.asarray(inputs["b_out2"], np.float32)

    N, TW = tweet.shape
    D = W_in.shape[0]
    OUT = W_out2.shape[1]
    assert N % C == 0 and TW % 128 == 0 and D == 128
    NPC = N // C
    NP2 = -(-NPC // WIN) * WIN
    src, dst = ei[0], ei[1]

    meta = _edge_meta(src, dst, et, N, NPC, NP2)
    shapes = dict(N=N, TW=TW, D=D, OUT=OUT, NPC=NPC, NP2=NP2)
    npdt = _np_dt()
    KT = TW // 128

    nc = _build_program(shapes, meta)

    # shared (replicated) weight tensors
    shared = {
        "wt": np.ascontiguousarray(
            W_tweet.reshape(KT, 128, 128).transpose(1, 0, 2)).astype(npdt),
        "bt": b_tweet.reshape(128, 1),
        "win": W_in.astype(npdt),
        "bin": b_in.reshape(128, 1),
        "wr": np.ascontiguousarray(
            rgcn_weight.transpose(1, 0, 2).reshape(128, R * 128)).astype(npdt),
        "root": rgcn_root.astype(npdt),
        "brg": rgcn_bias.reshape(128, 1),
        "w1": W_out1.astype(npdt),
        "b1": b_out1.reshape(128, 1),
        "w2": W_out2.astype(npdt),
        "b2": b_out2.reshape(OUT, 1),
        "iota": np.tile(np.arange(WIN, dtype=np.float32),
                        (128, 1)).astype(npdt),
    }

    in_maps = []
    for c in range(C):
        tw_c = np.zeros((KT, 128, NP2), npdt)
        tw_c[:, :, :NPC] = (tweet[c * NPC:(c + 1) * NPC].T
                            .reshape(KT, 128, NPC).astype(npdt))
        m = dict(shared)
        m["twT"] = tw_c
        m["idx16"] = meta["idxw"][c]
        m["mdst"] = meta["md"][c]
        m["mw"] = meta["mw"][c]
        in_maps.append(m)

    res = run_bass_kernel_spmd(nc, in_maps, core_ids=list(range(C)),
                               trace=TRACE, tmpdir=TMPDIR)
    LAST_RESULTS = res

    out = np.zeros((N, OUT), np.float32)
    for c in range(C):
        out[c * NPC:(c + 1) * NPC] = res.results[c]["outT"][:, :NPC].T
    return out
